# revision 1
# baseline (speedup 1.0000x reference)
"""Fused GQA attention block (QKV proj + RoPE + SDPA + out proj) on 8 TRN2
NeuronCores.

Sharding: tensor-parallel over heads. Core c owns kv-head c (q-heads
4c..4c+3): Wq/Wk/Wv column shards, Wo row shard. Each core computes a
full-shape partial of the output projection; the host sums the 8 partials.

All activations live in transposed [head_dim, token] layout on-chip; the
host pre-transposes X and the rope tables so no on-device transposition of
X is needed.  All matmuls run in float32r (fp32 storage, ~tf32-precision
matmul rounding, 1 PE cycle/row at moving-dim >= 256; measured rel err
~1.4e-4 at K=4096 vs 2.3e-3 for bf16).

Per-core dataflow:
  phase 1: Q^T/K^T/V^T = W^T X^T accumulated over D in 6 PSUM banks
           (Wq SBUF-resident, loaded once; X^T tiles streamed).  RoPE is
           applied via a +-1 rotation-matrix matmul on the PE
           (rotate-half, sign folded in) + two muls and an add on DVE.
           V^T is re-transposed to natural [token, hd] chunks via PE.
  phase 2: per (batch, q-head): S^T = K^T.T @ Q^T; P^T = exp(S^T*scale)
           on ACT straight out of PSUM; softmax denominators via a
           ones-matmul accumulated on the PE (gives the row-sum
           broadcast across partitions for free); O^T = V.T @ P^T,
           normalized by 1/l on DVE during the PSUM->SBUF copy.  No
           row-max subtraction: scores for this operator's input
           distribution are O(5), exp cannot overflow.
  phase 3: out_partial = O^T.T @ Wo shard (Wo SBUF-resident), streamed
           to DRAM; interleaved with phase 2 per (batch, q-half) group.
"""

from contextlib import ExitStack

import numpy as np

B, S, D = 2, 1024, 4096
HQ, HKV, HD = 32, 8, 128
NCORES = 8
QH = HQ // NCORES          # 4 q heads per core
MQ = QH * HD               # 512 q-projection columns per core
TT = B * S                 # 2048 tokens
P = 128
T5 = 512                   # token macro-tile
NT5 = TT // T5             # 4
ND = D // P                # 32 contraction chunks
SCALE = HD ** -0.5

_CACHE = {}
XT_BUFS = 10
ST_BUFS = 4
ACC_BUFS = 1
OUTP_BUFS = 2
ROPE_BUFS = 2


def _build_kernel(tc, out_ap, ins):
    from concourse import mybir

    nc = tc.nc
    F32 = mybir.dt.float32
    FP32R = mybir.dt.float32r
    Exp = mybir.ActivationFunctionType.Exp

    hst, cosT_d, sinT_d, wq, wk, wv, wo, consts = ins

    ctx = tc.ctx  # set by caller
    const = ctx.enter_context(tc.tile_pool(name="const", bufs=1))
    persist = ctx.enter_context(tc.tile_pool(name="persist", bufs=1))

    # ---- constants (identity, ones, rotation matrix) from DRAM ----------
    cc = const.tile([P, 3, P], F32)
    nc.sync.dma_start(cc.bitcast(FP32R), consts.bitcast(FP32R))
    ident = cc[:, 0]
    ones = cc[:, 1]
    rt = cc[:, 2]
    # ---- persistent activations -----------------------------------------
    qT = persist.tile([P, QH, TT], F32)        # Q^T per head
    kT = persist.tile([P, TT], F32)            # K^T (one kv head)
    vN = persist.tile([P, TT // P, P], F32)    # V natural [tok, hd] chunks
    oT = persist.tile([P, QH, TT], F32)        # attention out, transposed

    # ---- phases 0+1: cos/sin transpose, projections ---------------------
    wq_r = wq.rearrange("(o p) m -> p o m", p=P)   # [128, 32, 512]
    wk_r = wk.rearrange("(o p) m -> p o m", p=P)   # [128, 32, 128]
    wv_r = wv.rearrange("(o p) m -> p o m", p=P)

    with tc.tile_pool(name="ph1", bufs=1) as ph1, \
         tc.tile_pool(name="wpool", bufs=3) as wpool, \
         tc.tile_pool(name="xpool", bufs=6) as xpool, \
         tc.tile_pool(name="ropep", bufs=ROPE_BUFS) as ropep, \
         tc.tile_pool(name="proj_ps", bufs=6, space="PSUM") as proj_psum, \
         tc.tile_pool(name="tp_ps", bufs=2, space="PSUM") as tp_psum:
        wq_res = ph1.tile([P, ND, MQ], F32)   # Wq resident, chunk-loaded
        for t5 in range(NT5):
            tsl = slice(t5 * T5, (t5 + 1) * T5)
            projs = [proj_psum.tile([P, T5], F32, tag="proj", name=f"proj{i}")
                     for i in range(6)]
            for dJ in range(ND // 4):          # 8 macro chunks of 512 D
                dj4 = slice(dJ * 4, (dJ + 1) * 4)
                if t5 == 0:
                    nc.sync.dma_start(wq_res[:, dj4].bitcast(FP32R),
                                      wq_r[:, dj4].bitcast(FP32R))
                wq_sb = wq_res[:, dj4]
                wk_sb = wpool.tile([P, 4, HD], F32, tag="wk", name="wk_sb")
                nc.sync.dma_start(wk_sb.bitcast(FP32R), wk_r[:, dj4, :].bitcast(FP32R))
                wv_sb = wpool.tile([P, 4, HD], F32, tag="wv", name="wv_sb")
                nc.sync.dma_start(wv_sb.bitcast(FP32R), wv_r[:, dj4, :].bitcast(FP32R))
                for dj in range(4):
                    d = dJ * 4 + dj
                    xT = xpool.tile([P, T5], F32, tag="xT", bufs=XT_BUFS, name="xT")
                    nc.sync.dma_start(xT.bitcast(FP32R),
                                      hst[d * P:(d + 1) * P, tsl].bitcast(FP32R))
                    for oc in range(6):
                        if oc < QH:
                            w_sl = wq_sb[:, dj, oc * P:(oc + 1) * P]
                        elif oc == QH:
                            w_sl = wk_sb[:, dj, :]
                        else:
                            w_sl = wv_sb[:, dj, :]
                        nc.tensor.matmul(projs[oc][:], w_sl.bitcast(FP32R),
                                         xT.bitcast(FP32R),
                                         start=(d == 0), stop=(d == ND - 1))
            # epilogue: RoPE on Q (4 chunks) and K; V copy
            cosT = ropep.tile([P, T5], F32, tag="cosT", name="cosT")
            nc.sync.dma_start(cosT[:], cosT_d[:, tsl])
            sinT = ropep.tile([P, T5], F32, tag="sinT", name="sinT")
            nc.sync.dma_start(sinT[:], sinT_d[:, tsl])
            for oc in range(QH + 1):
                qraw = ropep.tile([P, T5], F32, tag="qraw", name="qraw")
                if oc % 2 == 0:
                    nc.scalar.copy(qraw.bitcast(FP32R), projs[oc][:])
                else:
                    nc.vector.tensor_copy(qraw.bitcast(FP32R), projs[oc][:])
                rot_ps = tp_psum.tile([P, T5], F32, tag="tp", name="rot_ps")
                nc.tensor.matmul(rot_ps[:], rt.bitcast(FP32R),
                                 qraw.bitcast(FP32R), start=True, stop=True)
                tmp = ropep.tile([P, T5], F32, tag="tmp", name="tmp")
                nc.vector.tensor_mul(tmp[:], rot_ps[:], sinT[:])
                tmp2 = ropep.tile([P, T5], F32, tag="tmp2", name="tmp2")
                nc.vector.tensor_mul(tmp2[:], qraw[:], cosT[:])
                dst = qT[:, oc, tsl] if oc < QH else kT[:, tsl]
                nc.vector.tensor_add(dst.bitcast(FP32R), tmp2[:], tmp[:])
            vtmp = ropep.tile([P, T5], F32, tag="vtmp", bufs=1, name="vtmp")
            nc.scalar.copy(vtmp.bitcast(FP32R), projs[QH + 1][:])
            v_ps = tp_psum.tile([P, T5], F32, tag="tp", name="v_ps")
            for i in range(4):
                nc.tensor.transpose(
                    v_ps[:, i * P:(i + 1) * P].bitcast(FP32R),
                    vtmp[:, i * P:(i + 1) * P].bitcast(FP32R),
                    ident.bitcast(FP32R))
            nc.scalar.copy(vN[:, t5 * 4:(t5 + 1) * 4, :].bitcast(FP32R),
                           v_ps[:])

    # ---- phases 2+3 interleaved: attention then out-proj per (b, qh) ----
    wo_r = wo.rearrange("(ho p) e -> p ho e", p=P)  # [128, 4, 4096]
    with tc.tile_pool(name="wopool", bufs=1) as wopool, \
         tc.tile_pool(name="attn", bufs=2) as apool, \
         tc.tile_pool(name="p_pool", bufs=6) as ppool, \
         tc.tile_pool(name="obuf", bufs=4) as obuf, \
         tc.tile_pool(name="st_ps", bufs=ST_BUFS, space="PSUM") as st_psum, \
         tc.tile_pool(name="acc_ps", bufs=ACC_BUFS, space="PSUM") as acc_psum, \
         tc.tile_pool(name="out_ps", bufs=OUTP_BUFS, space="PSUM") as out_psum:
        wo_sb = wopool.tile([P, QH, D], F32)       # resident Wo shard (8 MB)
        for ec in range(D // T5):
            esl = slice(ec * T5, (ec + 1) * T5)
            nc.sync.dma_start(wo_sb[:, :, esl].bitcast(FP32R),
                              wo_r[:, :, esl].bitcast(FP32R))
        for b in range(B):
            for qh in range(2):
                q0 = b * S + qh * T5
                qsl = slice(q0, q0 + T5)
                for h in range(QH):
                    oacc = acc_psum.tile([P, T5], F32, tag="oacc", name="oacc")
                    lacc = acc_psum.tile([P, T5], F32, tag="lacc", name="lacc")
                    # l-matmuls on DVE-pre-summed P^T pairs: halves the
                    # softmax-denominator matmul count on the PE
                    prev_p = None
                    for kc in range(S // P):
                        ksl = slice(b * S + kc * P, b * S + (kc + 1) * P)
                        st = st_psum.tile([P, T5], F32, tag="st", name="st")
                        nc.tensor.matmul(st[:], kT[:, ksl].bitcast(FP32R),
                                         qT[:, h, qsl].bitcast(FP32R),
                                         start=True, stop=True)
                        p_sb = ppool.tile([P, T5], F32, tag="p", name="p_sb")
                        nc.scalar.activation(p_sb.bitcast(FP32R), st[:], Exp,
                                             scale=SCALE)
                        nc.tensor.matmul(oacc[:],
                                         vN[:, b * (S // P) + kc, :].bitcast(FP32R),
                                         p_sb.bitcast(FP32R),
                                         start=(kc == 0), stop=(kc == S // P - 1))
                        if kc % 2 == 0:
                            prev_p = p_sb
                        else:
                            p_pair = ppool.tile([P, T5], F32, tag="pp",
                                                bufs=3, name="p_pair")
                            nc.vector.tensor_add(p_pair.bitcast(FP32R),
                                                 prev_p[:], p_sb[:])
                            nc.tensor.matmul(lacc[:], ones.bitcast(FP32R),
                                             p_pair.bitcast(FP32R),
                                             start=(kc == 1),
                                             stop=(kc == S // P - 1))
                    recip = apool.tile([P, T5], F32, tag="recip", name="recip")
                    nc.vector.reciprocal(recip[:], lacc[:])
                    nc.vector.tensor_mul(oT[:, h, qsl].bitcast(FP32R),
                                         oacc[:], recip[:])
                # out-proj for this token group (4 chunks of 128)
                for tcn in range(q0 // P, q0 // P + T5 // P):
                    obs_ = [obuf.tile([P, D // 2], F32, tag="ob", bufs=3,
                                      name="ob") for _ in range(2)]
                    for ec in range(D // T5):
                        ob = obs_[ec // 4]
                        esl = slice(ec * T5, (ec + 1) * T5)
                        out_ps = out_psum.tile([P, T5], F32, tag="outp",
                                               name="out_ps")
                        for hc in range(QH):
                            nc.tensor.matmul(
                                out_ps[:],
                                oT[:, hc, tcn * P:(tcn + 1) * P].bitcast(FP32R),
                                wo_sb[:, hc, esl].bitcast(FP32R),
                                start=(hc == 0), stop=(hc == QH - 1))
                        osl = slice((ec % 4) * T5, (ec % 4 + 1) * T5)
                        if ec % 2 == 0:
                            nc.vector.tensor_copy(ob[:, osl], out_ps[:])
                        else:
                            nc.scalar.copy(ob[:, osl], out_ps[:])
                    for half_i in range(2):
                        nc.sync.dma_start(
                            out_ap[tcn * P:(tcn + 1) * P,
                                   half_i * (D // 2):(half_i + 1) * (D // 2)],
                            obs_[half_i][:])



def _get_nc(nbody=1):
    key = ("nc", nbody)
    if key in _CACHE:
        return _CACHE[key]
    import concourse.tile as tile
    from concourse import bacc, mybir

    F32 = mybir.dt.float32
    nc = bacc.Bacc("TRN2", target_bir_lowering=False, debug=False)
    hst = nc.dram_tensor("hst", [D, TT], F32, kind="ExternalInput").ap()
    cost = nc.dram_tensor("cost", [HD, TT], F32, kind="ExternalInput").ap()
    sint = nc.dram_tensor("sint", [HD, TT], F32, kind="ExternalInput").ap()
    wq = nc.dram_tensor("wq", [D, MQ], F32, kind="ExternalInput").ap()
    wk = nc.dram_tensor("wk", [D, HD], F32, kind="ExternalInput").ap()
    wv = nc.dram_tensor("wv", [D, HD], F32, kind="ExternalInput").ap()
    wo = nc.dram_tensor("wo", [MQ, D], F32, kind="ExternalInput").ap()
    consts = nc.dram_tensor("consts", [P, 3 * P], F32, kind="ExternalInput").ap()
    out = nc.dram_tensor("out", [TT, D], F32, kind="ExternalOutput").ap()
    with tile.TileContext(nc) as tc:
        for _ in range(nbody):
            with ExitStack() as ctx:
                tc.ctx = ctx
                _build_kernel(tc, out, (hst, cost, sint, wq, wk, wv, wo,
                                        consts.rearrange('p (t q) -> p t q', t=3)))
    nc.compile()
    _CACHE[key] = nc
    return nc


def _in_maps(hidden_states, cos_table, sin_table, Wq, Wk, Wv, Wo):
    hst = np.ascontiguousarray(np.asarray(hidden_states, dtype=np.float32)
                               .reshape(TT, D).T)
    cost = np.ascontiguousarray(np.asarray(cos_table, dtype=np.float32)
                                .reshape(TT, HD).T)
    sint = np.ascontiguousarray(np.asarray(sin_table, dtype=np.float32)
                                .reshape(TT, HD).T)
    Wq = np.asarray(Wq, dtype=np.float32)
    Wk = np.asarray(Wk, dtype=np.float32)
    Wv = np.asarray(Wv, dtype=np.float32)
    Wo = np.asarray(Wo, dtype=np.float32)
    ident = np.eye(P, dtype=np.float32)
    ones = np.ones((P, P), dtype=np.float32)
    rt = np.zeros((P, P), dtype=np.float32)
    for k in range(64):
        rt[k, k + 64] = 1.0
    for k in range(64, P):
        rt[k, k - 64] = -1.0
    consts = np.concatenate([ident, ones, rt], axis=1)
    maps = []
    for c in range(NCORES):
        maps.append({
            "hst": hst,
            "cost": cost,
            "sint": sint,
            "wq": np.ascontiguousarray(Wq[:, c * MQ:(c + 1) * MQ]),
            "wk": np.ascontiguousarray(Wk[:, c * HD:(c + 1) * HD]),
            "wv": np.ascontiguousarray(Wv[:, c * HD:(c + 1) * HD]),
            "wo": np.ascontiguousarray(Wo[c * MQ:(c + 1) * MQ, :]),
            "consts": consts,
        })
    return maps


# inputs identical on every core: sent once and broadcast by shard_map
_REPLICATED = {"hst", "cost", "sint", "consts"}


def _get_runner(nbody=1):
    """Build the 8-core SPMD executable once (mirrors the multi-core branch
    of bass2jax.run_bass_via_pjrt, but cached so repeat calls don't re-jit
    or re-compile the NEFF).  Replicated inputs ship once; the zero output
    buffers the NEFF writes into are created on-device."""
    key = ("runner", nbody)
    if key in _CACHE:
        return _CACHE[key]
    import jax
    from jax.sharding import Mesh, PartitionSpec
    from jax.experimental.shard_map import shard_map
    import concourse.mybir as mybir
    from concourse import bass2jax

    nc = _get_nc(nbody)
    bass2jax.install_neuronx_cc_hook()

    part_name = nc.partition_id_tensor.name if nc.partition_id_tensor else None
    in_names, out_names, out_avals, zero_outs = [], [], [], []
    for alloc in nc.m.functions[0].allocations:
        if not isinstance(alloc, mybir.MemoryLocationSet):
            continue
        name = alloc.memorylocations[0].name
        if alloc.kind == "ExternalInput":
            if name != part_name:
                in_names.append(name)
        elif alloc.kind == "ExternalOutput":
            out_names.append(name)
            shape = tuple(alloc.tensor_shape)
            dtype = mybir.dt.np(alloc.dtype)
            out_avals.append(jax.core.ShapedArray(shape, dtype))
            zero_outs.append(np.zeros(shape, dtype))
    n_params = len(in_names)
    all_names = in_names + out_names
    if part_name is not None:
        all_names = all_names + [part_name]

    def _body(*args):
        operands = list(args)
        if part_name is not None:
            operands.append(bass2jax.partition_id_tensor())
        outs = bass2jax._bass_exec_p.bind(
            *operands,
            out_avals=tuple(out_avals),
            in_names=tuple(all_names),
            out_names=tuple(out_names),
            lowering_input_output_aliases=(),
            sim_require_finite=True,
            sim_require_nnan=True,
            nc=nc,
        )
        return tuple(outs)

    devices = jax.devices()[:NCORES]
    assert len(devices) == NCORES, (
        f"need {NCORES} NeuronCores, jax.devices() shows {len(jax.devices())}")
    mesh = Mesh(np.asarray(devices), ("core",))
    in_specs = tuple(PartitionSpec() if n in _REPLICATED
                     else PartitionSpec("core") for n in in_names) \
        + (PartitionSpec("core"),) * len(out_names)
    sharded = jax.jit(
        shard_map(_body, mesh=mesh,
                  in_specs=in_specs,
                  out_specs=(PartitionSpec("core"),) * len(out_names),
                  check_rep=False),
        keep_unused=True,
    )
    runner = (sharded, mesh, in_names, out_names, out_avals, zero_outs)
    _CACHE[key] = runner
    return runner


def _concat_inputs(maps):
    sharded, mesh, in_names, out_names, out_avals, zero_outs = _get_runner()
    concat_in = [maps[0][n] if n in _REPLICATED
                 else np.concatenate([maps[c][n] for c in range(NCORES)], axis=0)
                 for n in in_names]
    concat_zeros = [np.zeros((NCORES * z.shape[0], *z.shape[1:]), z.dtype)
                    for z in zero_outs]
    return concat_in + concat_zeros


def _run(maps):
    sharded, mesh, in_names, out_names, out_avals, zero_outs = _get_runner()
    out_arrs = sharded(*_concat_inputs(maps))
    return [np.asarray(out_arrs[0]).reshape(NCORES, *out_avals[0].shape)[c]
            for c in range(NCORES)]


def kernel(hidden_states, cos_table, sin_table, Wq, Wk, Wv, Wo):
    maps = _in_maps(hidden_states, cos_table, sin_table, Wq, Wk, Wv, Wo)
    parts = np.stack(_run(maps))
    out = parts.sum(axis=0, dtype=np.float64).astype(np.float32)
    return out.reshape(B, S, D)



# revision 19
# speedup vs baseline: 1.3903x; 1.3903x over previous
"""Fused GQA attention block (QKV proj + RoPE + SDPA + out proj) on 8 TRN2
NeuronCores.

Sharding: tensor-parallel over heads. Core c owns kv-head c (q-heads
4c..4c+3): Wq/Wk/Wv column shards, Wo row shard. Each core computes a
full-shape partial of the output projection; the host sums the 8 partials.

v2 vs baseline: all matmul operands and stored activations in bfloat16
(PSUM accumulation fp32), halving every DMA stream (X, weights, output
partials) and SBUF residency; Wo preloaded during phase 1; the softmax
denominator uses a DVE bf16 pair/tree reduction feeding ONE ones-matmul
per head (was 4); oacc PSUM double-buffered; AV matmuls shifted one slot
behind the score matmuls to hide the exp (ACT) latency; out-projection
software-pipelined one (batch, q-half) group behind attention.

Per-core dataflow:
  phase 1: Q^T/K^T/V^T = W^T X^T accumulated over D in 6 PSUM banks
           (bf16 weights SBUF-resident, X^T tiles streamed bf16).  RoPE
           applied via a +-1 rotation-matrix matmul on the PE (fp32r)
           + two muls and an add on DVE, writing qT/kT in bf16.
           V^T re-transposed to natural [token, hd] bf16 chunks via PE.
  phase 2: per (batch, q-head): S^T = K^T.T Q^T (bf16 in, fp32 PSUM);
           P^T = exp(S^T*scale) on ACT straight out of PSUM -> bf16;
           O^T = V.T P^T accumulated in PSUM; softmax denominators via
           a bf16 DVE pair/tree sum + one ones-matmul; O^T normalized
           by 1/l on DVE during the PSUM->SBUF copy (bf16 out).
  phase 3: out_partial = O^T.T @ Wo shard (bf16), streamed to DRAM as
           bf16 per-[128,512] chunks; runs one group behind attention.
"""

from contextlib import ExitStack

import numpy as np

B, S, D = 2, 1024, 4096
HQ, HKV, HD = 32, 8, 128
NCORES = 8
QH = HQ // NCORES          # 4 q heads per core
MQ = QH * HD               # 512 q-projection columns per core
TT = B * S                 # 2048 tokens
P = 128
T5 = 512                   # token macro-tile
NT5 = TT // T5             # 4
ND = D // P                # 32 contraction chunks
SCALE = HD ** -0.5
KC = S // P                # 8 key chunks per batch

_CACHE = {}
XT_BUFS = 8
ST_BUFS = 3


def _build_kernel(tc, out_ap, ins):
    from concourse import mybir

    nc = tc.nc
    F32 = mybir.dt.float32
    BF16 = mybir.dt.bfloat16
    FP32R = mybir.dt.float32r
    Exp = mybir.ActivationFunctionType.Exp

    hst, cosT_d, sinT_d, wq, wkv, wo, c16 = ins

    ctx = tc.ctx  # set by caller
    const = ctx.enter_context(tc.tile_pool(name="const", bufs=1))
    persist = ctx.enter_context(tc.tile_pool(name="persist", bufs=1))

    # ---- constants (loaded late: nothing needs them before the first
    # epilogue; keeps the startup DMA queue clear for weights + X) ---------
    cc16 = const.tile([P, 2, P], BF16)
    ident = cc16[:, 0]          # bf16 identity (V transpose)
    ones = cc16[:, 1]           # bf16 ones (softmax denominator matmul)
    def load_consts():
        nc.sync.dma_start(cc16, c16)

    # ---- persistent activations -----------------------------------------
    qT = persist.tile([P, QH, TT], BF16)        # Q^T per head
    kT = persist.tile([P, TT], BF16)            # K^T (one kv head)
    vN = persist.tile([P, TT // P, P], BF16)    # V natural [tok, hd] chunks
    oT = persist.tile([P, QH, TT], BF16)        # attention out, transposed
    wo_res = persist.tile([P, QH, D], BF16)     # resident Wo shard (4 MB)

    # ---- phases 0+1: projections + RoPE ---------------------------------
    wq_r = wq.rearrange("p (o m) -> p o m", o=ND)       # [128, 32, 512]
    wkv_r = wkv.rearrange("p (o k m) -> p o k m", o=ND, k=2)
    wo_r = wo.rearrange("p (h e) -> p h e", h=QH)   # [128, 4, 4096]

    with tc.tile_pool(name="ph1", bufs=1) as ph1, \
         tc.tile_pool(name="xpool", bufs=XT_BUFS) as xpool, \
         tc.tile_pool(name="ropep", bufs=2) as ropep, \
         tc.tile_pool(name="proj_ps", bufs=6, space="PSUM") as proj_psum, \
         tc.tile_pool(name="tp_ps", bufs=2, space="PSUM") as tp_psum:
        wq_res = ph1.tile([P, ND, MQ], BF16)
        wkv_res = ph1.tile([P, ND, 2, HD], BF16)
        for t5 in range(NT5):
            tsl = slice(t5 * T5, (t5 + 1) * T5)
            projs = [proj_psum.tile([P, T5], F32, tag="proj", name=f"proj{i}")
                     for i in range(6)]
            for dJ in range(ND // 4):          # 8 macro chunks of 512 D
                dj4 = slice(dJ * 4, (dJ + 1) * 4)
                if t5 == 0:
                    if dJ == 0:
                        # minimal first loads so compute starts sooner: only
                        # what the d=0 matmuls need, then the rest
                        nc.sync.dma_start(wq_res[:, 0:1], wq_r[:, 0:1])
                        nc.sync.dma_start(wkv_res[:, dj4], wkv_r[:, dj4])
                    else:
                        nc.sync.dma_start(wq_res[:, dj4], wq_r[:, dj4])
                        nc.sync.dma_start(wkv_res[:, dj4], wkv_r[:, dj4])
                for dj in range(4):
                    d = dJ * 4 + dj
                    xT = xpool.tile([P, T5], BF16, tag="xT", name="xT")
                    nc.sync.dma_start(xT, hst[d * P:(d + 1) * P, tsl])
                    if t5 == 0 and dJ == 0 and dj == 0:
                        nc.sync.dma_start(wq_res[:, 1:4], wq_r[:, 1:4])
                    if t5 == 0 and dJ == 1 and dj == 0:
                        load_consts()
                    for oc in range(6):
                        if oc < QH:
                            w_sl = wq_res[:, d, oc * P:(oc + 1) * P]
                        else:
                            w_sl = wkv_res[:, d, oc - QH, :]
                        nc.tensor.matmul(projs[oc][:], w_sl, xT,
                                         start=(d == 0), stop=(d == ND - 1))
            # spread the Wo preload across phase 1 (2 chunks per t5)
            for half in range(2):
                esl = slice((t5 * 2 + half) * T5, (t5 * 2 + half + 1) * T5)
                nc.sync.dma_start(wo_res[:, :, esl], wo_r[:, :, esl])
            # epilogue: RoPE on Q (4 chunks) and K; V transpose.  All PSUM
            # copies are hoisted first so the 6 proj banks free as early as
            # possible (the next t5's matmuls / phase-2 st tiles reuse them).
            cosT = ropep.tile([P, T5], F32, tag="cosT", name="cosT")
            nc.sync.dma_start(cosT[:], cosT_d[:, tsl])
            sinT = ropep.tile([P, T5], F32, tag="sinT", name="sinT")
            nc.sync.dma_start(sinT[:], sinT_d[:, tsl])
            qraws = []
            for oc in range(QH + 1):
                qraw = ropep.tile([P, T5], F32, tag="qraw", bufs=6,
                                  name="qraw")
                if oc % 2 == 0:
                    nc.scalar.copy(qraw[:], projs[oc][:])
                else:
                    nc.vector.tensor_copy(qraw[:], projs[oc][:])
                qraws.append(qraw)
            vtmp = ropep.tile([P, T5], BF16, tag="vtmp", bufs=1, name="vtmp")
            nc.scalar.copy(vtmp[:], projs[QH + 1][:])
            for oc in range(QH + 1):
                qraw = qraws[oc]
                # rotate-half via DMA partition swap; the sign lives in the
                # host-negated sin table (sneg = [-sin[0:64]; sin[64:128]])
                xsh = ropep.tile([P, T5], F32, tag="xsh", bufs=3, name="xsh")
                nc.sync.dma_start(xsh[0:HD // 2], qraw[HD // 2:P])
                nc.sync.dma_start(xsh[HD // 2:P], qraw[0:HD // 2])
                tmp = ropep.tile([P, T5], F32, tag="tmp", name="tmp")
                nc.vector.tensor_mul(tmp[:], xsh[:], sinT[:])
                tmp2 = ropep.tile([P, T5], F32, tag="tmp2", name="tmp2")
                nc.vector.tensor_mul(tmp2[:], qraw[:], cosT[:])
                dst = qT[:, oc, tsl] if oc < QH else kT[:, tsl]
                nc.vector.tensor_add(dst, tmp2[:], tmp[:])
            v_ps = tp_psum.tile([P, T5], BF16, tag="tp", name="v_ps")
            for i in range(4):
                nc.tensor.transpose(
                    v_ps[:, i * P:(i + 1) * P],
                    vtmp[:, i * P:(i + 1) * P],
                    ident)
            nc.scalar.copy(vN[:, t5 * 4:(t5 + 1) * 4, :], v_ps[:])

    # ---- phases 2+3: attention, out-proj one group behind ---------------
    with tc.tile_pool(name="attn", bufs=2) as apool, \
         tc.tile_pool(name="p_pool", bufs=6) as ppool, \
         tc.tile_pool(name="obuf", bufs=4) as obuf, \
         tc.tile_pool(name="st_ps", bufs=ST_BUFS, space="PSUM") as st_psum, \
         tc.tile_pool(name="oacc_ps", bufs=2, space="PSUM") as oacc_psum, \
         tc.tile_pool(name="lacc_ps", bufs=1, space="PSUM") as lacc_psum, \
         tc.tile_pool(name="out_ps", bufs=2, space="PSUM") as out_psum:

        def outproj_block(tcn, ec, last=False):
            # one [128 tok, 512 e] chunk of the previous group's out-proj:
            # 4 PE matmuls with no ACT dependency + a copy (ACT/DVE) + DMA
            esl = slice(ec * T5, (ec + 1) * T5)
            out_ps = out_psum.tile([P, T5], F32, tag="outp", name="out_ps")
            for hc in range(QH):
                nc.tensor.matmul(
                    out_ps[:],
                    oT[:, hc, tcn * P:(tcn + 1) * P],
                    wo_res[:, hc, esl],
                    start=(hc == 0), stop=(hc == QH - 1))
            ob = obuf.tile([P, T5], BF16, tag="ob", name="ob")
            tokens = slice(tcn * P, (tcn + 1) * P)
            if last:
                # final blocks: copy halves on both engines in parallel so
                # the single out-DMA can start sooner (shorter drain)
                h5 = T5 // 2
                nc.scalar.copy(ob[:, 0:h5], out_ps[:, 0:h5])
                nc.vector.tensor_copy(ob[:, h5:T5], out_ps[:, h5:T5])
                nc.sync.dma_start(out_ap[tokens, esl], ob[:])
                return
            if ec % 2 == 0:
                nc.scalar.copy(ob[:], out_ps[:])
            else:
                nc.vector.tensor_copy(ob[:], out_ps[:])
            nc.sync.dma_start(out_ap[tokens, esl], ob[:])

        def attention_group(b, qh, zip_blocks):
            # zip_blocks: iterator over the previous group's outproj blocks;
            # one is interleaved per kc step so the PE never starves while
            # ACT works through the exp backlog (exp 0.6us > st+av 0.43us)
            q0 = b * S + qh * T5
            qsl = slice(q0, q0 + T5)
            for h in range(QH):
                oacc = oacc_psum.tile([P, T5], F32, tag="oacc", name="oacc")
                lacc = lacc_psum.tile([P, T5], F32, tag="lacc", name="lacc")
                p_tiles = [None] * KC
                pp = [None] * (KC // 2)
                tt_ = [None] * 2
                for kc in range(KC):
                    ksl = slice(b * S + kc * P, b * S + (kc + 1) * P)
                    st = st_psum.tile([P, T5], F32, tag="st", name="st")
                    nc.tensor.matmul(st[:], kT[:, ksl], qT[:, h, qsl],
                                     start=True, stop=True)
                    p_sb = ppool.tile([P, T5], BF16, tag="p", bufs=16, name="p_sb")
                    nc.scalar.activation(p_sb[:], st[:], Exp, scale=SCALE)
                    p_tiles[kc] = p_sb
                    if kc % 2 == 1:
                        ppt = ppool.tile([P, T5], BF16, tag="pp", bufs=6,
                                         name="pp")
                        nc.vector.tensor_add(ppt[:], p_tiles[kc - 1][:],
                                             p_sb[:])
                        pp[kc // 2] = ppt
                    if kc == 3 or kc == KC - 1:
                        i = kc // 4
                        t_ = ppool.tile([P, T5], BF16, tag="tt", bufs=3,
                                        name="tt")
                        nc.vector.tensor_add(t_[:], pp[i * 2][:],
                                             pp[i * 2 + 1][:])
                        tt_[i] = t_
                    for blk in zip_blocks[:1]:
                        outproj_block(*blk)
                    del zip_blocks[:1]
                    if kc >= 1:
                        av = kc - 1
                        nc.tensor.matmul(oacc[:],
                                         vN[:, b * KC + av, :],
                                         p_tiles[av][:],
                                         start=(av == 0), stop=(av == KC - 1))
                nc.tensor.matmul(oacc[:], vN[:, b * KC + KC - 1, :],
                                 p_tiles[KC - 1][:], start=False, stop=True)
                ptot = ppool.tile([P, T5], BF16, tag="ptot", bufs=2,
                                  name="ptot")
                nc.vector.tensor_add(ptot[:], tt_[0][:], tt_[1][:])
                nc.tensor.matmul(lacc[:], ones, ptot[:], start=True, stop=True)
                recip = apool.tile([P, T5], F32, tag="recip", name="recip")
                for hf in range(2):
                    fsl = slice(hf * (T5 // 2), (hf + 1) * (T5 // 2))
                    nc.vector.reciprocal(recip[:, fsl], lacc[:, fsl])
                    nc.vector.tensor_mul(
                        oT[:, h, q0 + hf * (T5 // 2):q0 + (hf + 1) * (T5 // 2)],
                        oacc[:, fsl], recip[:, fsl])

        def group_blocks(b, qh):
            q0 = b * S + qh * T5
            return [(tcn, ec) for tcn in range(q0 // P, q0 // P + T5 // P)
                    for ec in range(D // T5)]

        groups = [(b, qh) for b in range(B) for qh in range(2)]
        for gi, (b, qh) in enumerate(groups):
            blocks = group_blocks(*groups[gi - 1]) if gi >= 1 else []
            attention_group(b, qh, blocks)
            for blk in blocks:      # any blocks not consumed by the zipper
                outproj_block(*blk)
        final_blocks = group_blocks(*groups[-1])
        for bi, blk in enumerate(final_blocks):
            outproj_block(*blk, last=(bi >= len(final_blocks) - 2))


def _get_nc(nbody=1):
    key = ("nc", nbody)
    if key in _CACHE:
        return _CACHE[key]
    import concourse.tile as tile
    from concourse import bacc, mybir

    F32 = mybir.dt.float32
    BF16 = mybir.dt.bfloat16
    nc = bacc.Bacc("TRN2", target_bir_lowering=False, debug=False)
    hst = nc.dram_tensor("hst", [D, TT], BF16, kind="ExternalInput").ap()
    cost = nc.dram_tensor("cost", [HD, TT], F32, kind="ExternalInput").ap()
    sint = nc.dram_tensor("sint", [HD, TT], F32, kind="ExternalInput").ap()
    wq = nc.dram_tensor("wq", [P, ND * MQ], BF16, kind="ExternalInput").ap()
    wkv = nc.dram_tensor("wkv", [P, ND * 2 * HD], BF16,
                         kind="ExternalInput").ap()
    wo = nc.dram_tensor("wo", [P, QH * D], BF16, kind="ExternalInput").ap()
    c16 = nc.dram_tensor("c16", [P, 2 * P], BF16, kind="ExternalInput").ap()
    out = nc.dram_tensor("out", [TT, D], BF16, kind="ExternalOutput").ap()
    with tile.TileContext(nc) as tc:
        for _ in range(nbody):
            with ExitStack() as ctx:
                tc.ctx = ctx
                _build_kernel(tc, out, (hst, cost, sint, wq, wkv, wo,
                                        c16.rearrange('p (t q) -> p t q',
                                                      t=2)))
    nc.compile()
    _CACHE[key] = nc
    return nc


def _bf16(x):
    import ml_dtypes
    return np.ascontiguousarray(
        np.asarray(x, dtype=np.float32).astype(ml_dtypes.bfloat16))


def _in_maps(hidden_states, cos_table, sin_table, Wq, Wk, Wv, Wo):
    hst = _bf16(np.asarray(hidden_states, dtype=np.float32)
                .reshape(TT, D).T)
    cost = np.ascontiguousarray(np.asarray(cos_table, dtype=np.float32)
                                .reshape(TT, HD).T)
    sint = np.ascontiguousarray(np.asarray(sin_table, dtype=np.float32)
                                .reshape(TT, HD).T)
    # rotate-half sign folded into the sin table: rows 0..63 negated
    sint[:HD // 2] *= -1.0
    Wq = np.asarray(Wq, dtype=np.float32)
    Wk = np.asarray(Wk, dtype=np.float32)
    Wv = np.asarray(Wv, dtype=np.float32)
    Wo = np.asarray(Wo, dtype=np.float32)
    ident = np.eye(P, dtype=np.float32)
    ones = np.ones((P, P), dtype=np.float32)
    c16 = _bf16(np.concatenate([ident, ones], axis=1))

    def _rearr(w, m):
        # [(o p), m] -> [p, o*m] partition-major for big contiguous DMA rows
        o = w.shape[0] // P
        return _bf16(w.reshape(o, P, m).transpose(1, 0, 2).reshape(P, o * m))

    maps = []
    for c in range(NCORES):
        wk_s = Wk[:, c * HD:(c + 1) * HD].reshape(ND, P, 1, HD)
        wv_s = Wv[:, c * HD:(c + 1) * HD].reshape(ND, P, 1, HD)
        wkv = np.concatenate([wk_s, wv_s], axis=2)       # [o, p, 2, m]
        wkv = _bf16(wkv.transpose(1, 0, 2, 3).reshape(P, ND * 2 * HD))
        maps.append({
            "hst": hst,
            "cost": cost,
            "sint": sint,
            "wq": _rearr(Wq[:, c * MQ:(c + 1) * MQ], MQ),
            "wkv": wkv,
            "wo": _rearr(Wo[c * MQ:(c + 1) * MQ, :], D),
            "c16": c16,
        })
    return maps


# inputs identical on every core: sent once and broadcast by shard_map
_REPLICATED = {"hst", "cost", "sint", "c16"}


def _get_runner(nbody=1):
    """Build the 8-core SPMD executable once (mirrors the multi-core branch
    of bass2jax.run_bass_via_pjrt, but cached so repeat calls don't re-jit
    or re-compile the NEFF).  Replicated inputs ship once; the zero output
    buffers the NEFF writes into are created on-device."""
    key = ("runner", nbody)
    if key in _CACHE:
        return _CACHE[key]
    import jax
    from jax.sharding import Mesh, PartitionSpec
    from jax.experimental.shard_map import shard_map
    import concourse.mybir as mybir
    from concourse import bass2jax

    nc = _get_nc(nbody)
    bass2jax.install_neuronx_cc_hook()

    part_name = nc.partition_id_tensor.name if nc.partition_id_tensor else None
    in_names, out_names, out_avals, zero_outs = [], [], [], []
    for alloc in nc.m.functions[0].allocations:
        if not isinstance(alloc, mybir.MemoryLocationSet):
            continue
        name = alloc.memorylocations[0].name
        if alloc.kind == "ExternalInput":
            if name != part_name:
                in_names.append(name)
        elif alloc.kind == "ExternalOutput":
            out_names.append(name)
            shape = tuple(alloc.tensor_shape)
            dtype = mybir.dt.np(alloc.dtype)
            out_avals.append(jax.core.ShapedArray(shape, dtype))
            zero_outs.append(np.zeros(shape, dtype))
    n_params = len(in_names)
    all_names = in_names + out_names
    if part_name is not None:
        all_names = all_names + [part_name]

    def _body(*args):
        operands = list(args)
        if part_name is not None:
            operands.append(bass2jax.partition_id_tensor())
        outs = bass2jax._bass_exec_p.bind(
            *operands,
            out_avals=tuple(out_avals),
            in_names=tuple(all_names),
            out_names=tuple(out_names),
            lowering_input_output_aliases=(),
            sim_require_finite=True,
            sim_require_nnan=True,
            nc=nc,
        )
        return tuple(outs)

    devices = jax.devices()[:NCORES]
    assert len(devices) == NCORES, (
        f"need {NCORES} NeuronCores, jax.devices() shows {len(jax.devices())}")
    mesh = Mesh(np.asarray(devices), ("core",))
    in_specs = tuple(PartitionSpec() if n in _REPLICATED
                     else PartitionSpec("core") for n in in_names) \
        + (PartitionSpec("core"),) * len(out_names)
    sharded = jax.jit(
        shard_map(_body, mesh=mesh,
                  in_specs=in_specs,
                  out_specs=(PartitionSpec("core"),) * len(out_names),
                  check_rep=False),
        keep_unused=True,
    )
    runner = (sharded, mesh, in_names, out_names, out_avals, zero_outs)
    _CACHE[key] = runner
    return runner


def _concat_inputs(maps):
    sharded, mesh, in_names, out_names, out_avals, zero_outs = _get_runner()
    concat_in = [maps[0][n] if n in _REPLICATED
                 else np.concatenate([maps[c][n] for c in range(NCORES)], axis=0)
                 for n in in_names]
    concat_zeros = [np.zeros((NCORES * z.shape[0], *z.shape[1:]), z.dtype)
                    for z in zero_outs]
    return concat_in + concat_zeros


def _run(maps):
    sharded, mesh, in_names, out_names, out_avals, zero_outs = _get_runner()
    out_arrs = sharded(*_concat_inputs(maps))
    return [np.asarray(out_arrs[0]).reshape(NCORES, *out_avals[0].shape)[c]
            for c in range(NCORES)]


def kernel(hidden_states, cos_table, sin_table, Wq, Wk, Wv, Wo):
    maps = _in_maps(hidden_states, cos_table, sin_table, Wq, Wk, Wv, Wo)
    parts = np.stack([p.astype(np.float32) for p in _run(maps)])
    out = parts.sum(axis=0, dtype=np.float32)
    return out.reshape(B, S, D)
